# revision 1
# baseline (speedup 1.0000x reference)
"""Trainium2 Bass kernel for nn_CBF (dense MLP forward + Jacobian row).

Math: the reference computes HJH = [h, Jh] where
  h  = MLP(x_norm)            (scalar)
  Jh = Wout @ W3 @ D2 @ W2 @ D1 @ W1 @ D0 @ (W0 / x_range)   (1 x n row)
Everything is linear in the Jacobian chain, so Jh collapses to
  r0 @ W0 / x_range   with  r0 = left-chain of tiny 128-dim vectors.
Only two passes over the big W0 (128 x 131072) are needed:
  matvec1: V0 = x_norm @ W0.T    (contraction over n)
  matvec2: J  = r0 @ W0          (contraction over h)
W0 is sharded along n over 8 cores (16K cols, 8.4 MB per core per pass).

Two kernel launches (a cross-core AllReduce measured ~114 us on this stack,
vs ~40 us for a full slab DMA, so shuttling 4 KB of partials through the
host between launches is much cheaper):
  Launch A: per-core transposed slab W0T [n-chunk k, c, h]; matvec1 as 128
    PSUM-accumulated matmuls (lhsT = W0T chunk [k, h], rhs = x_norm column),
    fully hidden under the slab DMA. Returns V0 partials [128, 1] per core.
  Host: concatenates the 8 partials (pure gather, no arithmetic).
  Launch B: per-core natural slab W0 [h, n]; reduces partials on-device,
    runs the tiny forward/backward tanh chain on PE/ACT/DVE in column space,
    then matvec2 as 128 stationary-weight matmuls (lhsT = W0 chunk [h, k],
    rhs = r0 column) pipelined with the slab DMA. Scales by 1/x_range and
    returns the Jacobian slab [k, c] plus the scalar h.
"""

import os
import sys

import numpy as np

sys.path.insert(0, "/opt/trn_rl_repo")

import concourse.tile as tile  # noqa: E402
from concourse import bacc, mybir  # noqa: E402
from concourse import bass_utils  # noqa: E402

N_STATE = 131072
H = 128
N_CORES = 8
N_LOC = N_STATE // N_CORES  # 16384
C = N_LOC // 128  # 128 chunks of 128 per core
W0_TILE = 2048
N_W0_TILES = N_LOC // W0_TILE  # 8
CH_PER_TILE = W0_TILE // 128  # 16

FP = mybir.dt.float32
AOT = mybir.AluOpType
ACT = mybir.ActivationFunctionType

_CACHE = {}


def _build_a():
    """Launch A: transposed slab, matvec1 partials."""
    nc = bacc.Bacc("TRN2", target_bir_lowering=False, debug=False,
                   num_devices=N_CORES)

    w0t_d = nc.dram_tensor("w0t", [128, N_LOC], FP, kind="ExternalInput").ap()
    xsT_d = nc.dram_tensor("xsT", [128, C], FP, kind="ExternalInput").ap()
    xmaxT_d = nc.dram_tensor("xmaxT", [128, C], FP, kind="ExternalInput").ap()
    xminT_d = nc.dram_tensor("xminT", [128, C], FP, kind="ExternalInput").ap()
    outp_d = nc.dram_tensor("out_p", [H, 1], FP, kind="ExternalOutput").ap()

    with tile.TileContext(nc) as tc:
        with tc.tile_pool(name="w0", bufs=N_W0_TILES) as w0p, \
             tc.tile_pool(name="small", bufs=1) as sp, \
             tc.tile_pool(name="psl", bufs=1, space="PSUM") as plp:

            xsT = sp.tile([128, C], FP)
            nc.sync.dma_start(xsT[:], xsT_d[:])
            xmaxT = sp.tile([128, C], FP)
            nc.sync.dma_start(xmaxT[:], xmaxT_d[:])
            xminT = sp.tile([128, C], FP)
            nc.sync.dma_start(xminT[:], xminT_d[:])

            # x_norm = (state - (max+min)/2) * (2/(max-min)), in [k, c] layout
            xcT = sp.tile([128, C], FP)
            nc.vector.tensor_add(xcT[:], xmaxT[:], xminT[:])
            nc.vector.tensor_scalar_mul(xcT[:], xcT[:], 0.5)
            xrT = sp.tile([128, C], FP)
            nc.vector.tensor_sub(xrT[:], xmaxT[:], xminT[:])
            nc.vector.tensor_scalar_mul(xrT[:], xrT[:], 0.5)
            invT = sp.tile([128, C], FP)
            nc.vector.reciprocal(invT[:], xrT[:])
            xnT = sp.tile([128, C], FP)
            nc.vector.tensor_sub(xnT[:], xsT[:], xcT[:])
            nc.vector.tensor_mul(xnT[:], xnT[:], invT[:])

            v0ps = plp.tile([H, 1], FP)
            for t in range(N_W0_TILES):
                w0tile = w0p.tile([128, W0_TILE], FP, tag="w0tile")
                eng = nc.sync if t % 2 == 0 else nc.scalar
                eng.dma_start(w0tile[:], w0t_d[:, t * W0_TILE:(t + 1) * W0_TILE])
                for cc in range(CH_PER_TILE):
                    c = t * CH_PER_TILE + cc
                    nc.tensor.matmul(
                        v0ps[:],
                        w0tile[:, cc * 128:(cc + 1) * 128],
                        xnT[:, c:c + 1],
                        start=(c == 0),
                        stop=(c == C - 1),
                    )

            v0sb = sp.tile([H, 1], FP)
            nc.vector.tensor_copy(v0sb[:], v0ps[:])
            nc.sync.dma_start(outp_d[:], v0sb[:])

    nc.compile()
    return nc


def _build_b():
    """Launch B: natural slab, partial-reduce + chain + matvec2."""
    nc = bacc.Bacc("TRN2", target_bir_lowering=False, debug=False,
                   num_devices=N_CORES)

    w0_d = nc.dram_tensor("w0", [H, N_LOC], FP, kind="ExternalInput").ap()
    parts_d = nc.dram_tensor("parts", [H, N_CORES], FP, kind="ExternalInput").ap()
    xmaxT_d = nc.dram_tensor("xmaxT", [128, C], FP, kind="ExternalInput").ap()
    xminT_d = nc.dram_tensor("xminT", [128, C], FP, kind="ExternalInput").ap()
    w1t_d = nc.dram_tensor("w1t", [H, H], FP, kind="ExternalInput").ap()
    w2t_d = nc.dram_tensor("w2t", [H, H], FP, kind="ExternalInput").ap()
    w3t_d = nc.dram_tensor("w3t", [H, H], FP, kind="ExternalInput").ap()
    w1n_d = nc.dram_tensor("w1n", [H, H], FP, kind="ExternalInput").ap()
    w2n_d = nc.dram_tensor("w2n", [H, H], FP, kind="ExternalInput").ap()
    w3n_d = nc.dram_tensor("w3n", [H, H], FP, kind="ExternalInput").ap()
    # bcols columns: 0=b0 1=b1 2=b2 3=b3 4=Wout.T
    bcols_d = nc.dram_tensor("bcols", [H, 8], FP, kind="ExternalInput").ap()
    bout_d = nc.dram_tensor("bout", [1, 1], FP, kind="ExternalInput").ap()

    outj_d = nc.dram_tensor("out_j", [128, C], FP, kind="ExternalOutput").ap()
    outv_d = nc.dram_tensor("out_v", [1, 1], FP, kind="ExternalOutput").ap()

    with tile.TileContext(nc) as tc:
        with tc.tile_pool(name="w0", bufs=N_W0_TILES) as w0p, \
             tc.tile_pool(name="small", bufs=1) as sp, \
             tc.tile_pool(name="ps", bufs=2, space="PSUM") as pp, \
             tc.tile_pool(name="psj", bufs=1, space="PSUM") as pjp:

            # small loads
            parts = sp.tile([H, N_CORES], FP)
            nc.sync.dma_start(parts[:], parts_d[:])
            xmaxT = sp.tile([128, C], FP)
            nc.sync.dma_start(xmaxT[:], xmaxT_d[:])
            xminT = sp.tile([128, C], FP)
            nc.sync.dma_start(xminT[:], xminT_d[:])
            w1t = sp.tile([H, H], FP)
            nc.sync.dma_start(w1t[:], w1t_d[:])
            w2t = sp.tile([H, H], FP)
            nc.sync.dma_start(w2t[:], w2t_d[:])
            w3t = sp.tile([H, H], FP)
            nc.sync.dma_start(w3t[:], w3t_d[:])
            w1n = sp.tile([H, H], FP)
            nc.sync.dma_start(w1n[:], w1n_d[:])
            w2n = sp.tile([H, H], FP)
            nc.sync.dma_start(w2n[:], w2n_d[:])
            w3n = sp.tile([H, H], FP)
            nc.sync.dma_start(w3n[:], w3n_d[:])
            bcols = sp.tile([H, 8], FP)
            nc.sync.dma_start(bcols[:], bcols_d[:])
            boutt = sp.tile([1, 1], FP)
            nc.sync.dma_start(boutt[:], bout_d[:])
            one11 = sp.tile([1, 1], FP)
            nc.vector.memset(one11[:], 1.0)

            # 1/x_range in [k, c] layout
            xrT = sp.tile([128, C], FP)
            nc.vector.tensor_sub(xrT[:], xmaxT[:], xminT[:])
            nc.vector.tensor_scalar_mul(xrT[:], xrT[:], 0.5)
            invT = sp.tile([128, C], FP)
            nc.vector.reciprocal(invT[:], xrT[:])

            # ---- chain (all vectors as [128, 1] columns) ----
            v0c = sp.tile([H, 1], FP)
            nc.vector.tensor_reduce(v0c[:], parts[:], mybir.AxisListType.X, AOT.add)

            v1c = sp.tile([H, 1], FP)
            nc.scalar.activation(v1c[:], v0c[:], ACT.Tanh, bias=bcols[:, 0:1])
            d0c = sp.tile([H, 1], FP)
            nc.vector.tensor_mul(d0c[:], v1c[:], v1c[:])
            nc.vector.tensor_scalar(d0c[:], d0c[:], -1.0, 1.0, AOT.mult, AOT.add)

            a1ps = pp.tile([H, 1], FP, tag="chain_ps")
            nc.tensor.matmul(a1ps[:], w1t[:], v1c[:], start=True, stop=True)
            v2c = sp.tile([H, 1], FP)
            nc.scalar.activation(v2c[:], a1ps[:], ACT.Tanh, bias=bcols[:, 1:2])
            d1c = sp.tile([H, 1], FP)
            nc.vector.tensor_mul(d1c[:], v2c[:], v2c[:])
            nc.vector.tensor_scalar(d1c[:], d1c[:], -1.0, 1.0, AOT.mult, AOT.add)

            a2ps = pp.tile([H, 1], FP, tag="chain_ps")
            nc.tensor.matmul(a2ps[:], w2t[:], v2c[:], start=True, stop=True)
            v3c = sp.tile([H, 1], FP)
            nc.scalar.activation(v3c[:], a2ps[:], ACT.Tanh, bias=bcols[:, 2:3])
            d2c = sp.tile([H, 1], FP)
            nc.vector.tensor_mul(d2c[:], v3c[:], v3c[:])
            nc.vector.tensor_scalar(d2c[:], d2c[:], -1.0, 1.0, AOT.mult, AOT.add)

            a3ps = pp.tile([H, 1], FP, tag="chain_ps")
            nc.tensor.matmul(a3ps[:], w3t[:], v3c[:], start=True, stop=True)
            a3c = sp.tile([H, 1], FP)
            nc.scalar.activation(a3c[:], a3ps[:], ACT.Identity, bias=bcols[:, 3:4])

            voutps = pp.tile([1, 1], FP, tag="chain_ps")
            nc.tensor.matmul(voutps[:], bcols[:, 4:5], a3c[:], start=True, stop=True)
            voutsb = sp.tile([1, 1], FP)
            nc.scalar.activation(voutsb[:], voutps[:], ACT.Identity, bias=boutt[:])
            nc.sync.dma_start(outv_d[:], voutsb[:])

            # backward: r3 = Wout @ W3; q2 = r3*d2; r2 = q2 @ W2; q1 = r2*d1;
            # r1 = q1 @ W1; r0 = r1*d0
            r3rp = pp.tile([1, H], FP, tag="chain_ps")
            nc.tensor.matmul(r3rp[:], bcols[:, 4:5], w3n[:], start=True, stop=True)
            r3r = sp.tile([1, H], FP)
            nc.vector.tensor_copy(r3r[:], r3rp[:])
            r3cp = pp.tile([H, 1], FP, tag="chain_ps")
            nc.tensor.matmul(r3cp[:], r3r[:], one11[:], start=True, stop=True)
            q2c = sp.tile([H, 1], FP)
            nc.vector.tensor_mul(q2c[:], r3cp[:], d2c[:])

            r2rp = pp.tile([1, H], FP, tag="chain_ps")
            nc.tensor.matmul(r2rp[:], q2c[:], w2n[:], start=True, stop=True)
            r2r = sp.tile([1, H], FP)
            nc.vector.tensor_copy(r2r[:], r2rp[:])
            r2cp = pp.tile([H, 1], FP, tag="chain_ps")
            nc.tensor.matmul(r2cp[:], r2r[:], one11[:], start=True, stop=True)
            q1c = sp.tile([H, 1], FP)
            nc.vector.tensor_mul(q1c[:], r2cp[:], d1c[:])

            r1rp = pp.tile([1, H], FP, tag="chain_ps")
            nc.tensor.matmul(r1rp[:], q1c[:], w1n[:], start=True, stop=True)
            r1r = sp.tile([1, H], FP)
            nc.vector.tensor_copy(r1r[:], r1rp[:])
            r1cp = pp.tile([H, 1], FP, tag="chain_ps")
            nc.tensor.matmul(r1cp[:], r1r[:], one11[:], start=True, stop=True)
            r0c = sp.tile([H, 1], FP)
            nc.vector.tensor_mul(r0c[:], r1cp[:], d0c[:])

            # ---- matvec2: J[c*128+k] = sum_h W0[h, c*128+k] * r0[h] ----
            jt = pjp.tile([128, C], FP)
            for t in range(N_W0_TILES):
                w0tile = w0p.tile([128, W0_TILE], FP, tag="w0tile")
                eng = nc.sync if t % 2 == 0 else nc.scalar
                eng.dma_start(w0tile[:], w0_d[:, t * W0_TILE:(t + 1) * W0_TILE])
                for cc in range(CH_PER_TILE):
                    c = t * CH_PER_TILE + cc
                    nc.tensor.matmul(
                        jt[:, c:c + 1],
                        w0tile[:, cc * 128:(cc + 1) * 128],
                        r0c[:],
                        start=True,
                        stop=True,
                    )

            jts = sp.tile([128, C], FP)
            nc.vector.tensor_mul(jts[:], jt[:], invT[:])
            nc.sync.dma_start(outj_d[:], jts[:])

    nc.compile()
    return nc


def _get_kernels():
    if "nc_a" not in _CACHE:
        _CACHE["nc_a"] = _build_a()
        _CACHE["nc_b"] = _build_b()
    return _CACHE["nc_a"], _CACHE["nc_b"]


def kernel(**inputs):
    nc_a, nc_b = _get_kernels()
    f = np.float32

    state = np.asarray(inputs["state"], f).reshape(1, N_STATE)
    x_max = np.asarray(inputs["x_max"], f).reshape(N_STATE)
    x_min = np.asarray(inputs["x_min"], f).reshape(N_STATE)
    W0 = np.asarray(inputs["W0"], f)
    W1 = np.asarray(inputs["W1"], f)
    W2 = np.asarray(inputs["W2"], f)
    W3 = np.asarray(inputs["W3"], f)
    Wout = np.asarray(inputs["Wout"], f).reshape(1, H)
    b0 = np.asarray(inputs["b0"], f).reshape(H)
    b1 = np.asarray(inputs["b1"], f).reshape(H)
    b2 = np.asarray(inputs["b2"], f).reshape(H)
    b3 = np.asarray(inputs["b3"], f).reshape(H)
    bout = np.asarray(inputs["bout"], f).reshape(1)

    xmaxT = []
    xminT = []
    in_maps_a = []
    for i in range(N_CORES):
        sl = slice(i * N_LOC, (i + 1) * N_LOC)
        w0t = np.ascontiguousarray(
            W0[:, sl].reshape(H, C, 128).transpose(2, 1, 0)
        ).reshape(128, C * H)
        xmaxT.append(np.ascontiguousarray(x_max[sl].reshape(C, 128).T))
        xminT.append(np.ascontiguousarray(x_min[sl].reshape(C, 128).T))
        in_maps_a.append({
            "w0t": w0t,
            "xsT": np.ascontiguousarray(state[0, sl].reshape(C, 128).T),
            "xmaxT": xmaxT[i],
            "xminT": xminT[i],
        })

    trace = bool(int(os.environ.get("KERNEL_TRACE", "0")))
    res_a = bass_utils.run_bass_kernel_spmd(
        nc_a, in_maps_a, core_ids=list(range(N_CORES)), trace=trace
    )
    _CACHE["res_a"] = res_a

    # pure gather: stack the 8 per-core partial columns
    parts = np.ascontiguousarray(
        np.concatenate([res_a.results[i]["out_p"] for i in range(N_CORES)], axis=1)
    )

    bcolsm = np.zeros((H, 8), f)
    bcolsm[:, 0] = b0
    bcolsm[:, 1] = b1
    bcolsm[:, 2] = b2
    bcolsm[:, 3] = b3
    bcolsm[:, 4] = Wout[0]
    shared_b = {
        "parts": parts,
        "w1t": np.ascontiguousarray(W1.T),
        "w2t": np.ascontiguousarray(W2.T),
        "w3t": np.ascontiguousarray(W3.T),
        "w1n": np.ascontiguousarray(W1),
        "w2n": np.ascontiguousarray(W2),
        "w3n": np.ascontiguousarray(W3),
        "bcols": bcolsm,
        "bout": np.ascontiguousarray(bout.reshape(1, 1)),
    }
    in_maps_b = []
    for i in range(N_CORES):
        sl = slice(i * N_LOC, (i + 1) * N_LOC)
        m = {
            "w0": np.ascontiguousarray(W0[:, sl]),
            "xmaxT": xmaxT[i],
            "xminT": xminT[i],
        }
        m.update(shared_b)
        in_maps_b.append(m)

    res_b = bass_utils.run_bass_kernel_spmd(
        nc_b, in_maps_b, core_ids=list(range(N_CORES)), trace=trace
    )
    _CACHE["res_b"] = res_b

    out = np.empty((1, N_STATE + 1), np.float32)
    out[0, 0] = float(np.asarray(res_b.results[0]["out_v"]).reshape(()))
    for i in range(N_CORES):
        jt = np.asarray(res_b.results[i]["out_j"])  # [k, c]
        out[0, 1 + i * N_LOC:1 + (i + 1) * N_LOC] = jt.T.reshape(-1)
    return out



# revision 2
# speedup vs baseline: 3.0741x; 3.0741x over previous
"""Trainium2 Bass kernel for nn_CBF (dense MLP forward + Jacobian row).

Math: HJH = [h, Jh] with h = MLP(x_norm) (scalar) and
  Jh = Wout @ W3 @ D2 @ W2 @ D1 @ W1 @ D0 @ (W0 / x_range)  (1 x n row).
The Jacobian chain collapses to r0 @ W0 / x_range with r0 a 128-vector,
so only two passes over the big W0 (128 x 131072) are needed:
  pass 1: V0 = x_norm @ W0.T   (contract over n)
  pass 2: J  = r0 @ W0         (contract over h)
W0 is sharded along n over 8 cores (16K cols/core/pass).

v2 vs baseline (173.9us):
  * W0 slabs are cast to fp16 on the host: halves the HBM traffic
    (4.2MB/core/pass) and makes each matmul single-pass (fp32 matmuls
    are double-pumped LOW/HIGH on TRN2, doubling LDWEIGHTS+MATMUL).
  * pass 1 swaps matmul operands: the tiny x_norm column is the
    stationary (LDWEIGHTS of 1 column ~ 1 cycle) and the W0T chunk is
    the moving operand (128 cols/chunk). The baseline loaded a fresh
    128x128 stationary per chunk: 2x128 LDW columns at 1.2GHz = 427ns
    per chunk, i.e. a 55us serial PE chain - the measured bottleneck.
  * all small inputs packed into ONE DMA per launch (issue cost on the
    sync queue was ~0.64us per dma_start; baseline had 12-17 of them
    gating the first matmul at t=20us).
  * backward chain uses lhsT=natural weights to produce column vectors
    directly (baseline did row-matmul + transpose-matmul pairs).
Still two launches: the 8 partial V0 columns must be summed across
cores before the tanh chain; shuttling 512B/core through the host
between launches is far cheaper than an on-device AllReduce here.
"""

import os
import sys

import numpy as np

sys.path.insert(0, "/opt/trn_rl_repo")

import concourse.tile as tile  # noqa: E402
from concourse import bacc, mybir  # noqa: E402
from concourse import bass_utils  # noqa: E402

N_STATE = 131072
H = 128
N_CORES = 8
N_LOC = N_STATE // N_CORES  # 16384
C = N_LOC // 128  # 128 chunks of 128 per core
W0_TILE = 2048
N_W0_TILES = N_LOC // W0_TILE  # 8
CH_PER_TILE = W0_TILE // 128  # 16

FP = mybir.dt.float32
FH = mybir.dt.float16
AOT = mybir.AluOpType
ACT = mybir.ActivationFunctionType

_CACHE = {}


def _build_a():
    """Launch A: fp16 transposed slab; V0 partial = x_norm @ W0T."""
    nc = bacc.Bacc("TRN2", target_bir_lowering=False, debug=False,
                   num_devices=N_CORES)

    w0t_d = nc.dram_tensor("w0t", [128, N_LOC], FH, kind="ExternalInput").ap()
    # packed: cols 0:128 xsT | 128:256 xmaxT | 256:384 xminT  (all [k, c])
    sm_d = nc.dram_tensor("sm", [128, 384], FP, kind="ExternalInput").ap()
    outp_d = nc.dram_tensor("out_p", [1, H], FP, kind="ExternalOutput").ap()

    with tile.TileContext(nc) as tc:
        with tc.tile_pool(name="w0", bufs=N_W0_TILES) as w0p, \
             tc.tile_pool(name="small", bufs=1) as sp, \
             tc.tile_pool(name="psl", bufs=1, space="PSUM") as plp:

            sm = sp.tile([128, 384], FP)
            nc.sync.dma_start(sm[:], sm_d[:])
            w0tiles = []
            for t in range(N_W0_TILES):
                w0tile = w0p.tile([128, W0_TILE], FH, tag="w0tile")
                eng = nc.scalar if t % 2 == 0 else nc.sync
                eng.dma_start(w0tile[:], w0t_d[:, t * W0_TILE:(t + 1) * W0_TILE])
                w0tiles.append(w0tile)

            xsT = sm[:, 0:128]
            xmaxT = sm[:, 128:256]
            xminT = sm[:, 256:384]
            # x_norm = (state - (max+min)/2) * (2/(max-min)), in [k, c] layout
            xcT = sp.tile([128, C], FP)
            nc.vector.tensor_add(xcT[:], xmaxT, xminT)
            nc.vector.tensor_scalar_mul(xcT[:], xcT[:], 0.5)
            xrT = sp.tile([128, C], FP)
            nc.vector.tensor_sub(xrT[:], xmaxT, xminT)
            nc.vector.tensor_scalar_mul(xrT[:], xrT[:], 0.5)
            invT = sp.tile([128, C], FP)
            nc.vector.reciprocal(invT[:], xrT[:])
            xnT = sp.tile([128, C], FP)
            nc.vector.tensor_sub(xnT[:], xsT, xcT[:])
            nc.vector.tensor_mul(xnT[:], xnT[:], invT[:])
            xn16 = sp.tile([128, C], FH)
            nc.vector.tensor_copy(xn16[:], xnT[:])

            # V0[1, h] += xn_c.T @ W0T_chunk_c  -- stationary is 1 column
            v0ps = plp.tile([1, H], FP)
            for t in range(N_W0_TILES):
                for cc in range(CH_PER_TILE):
                    c = t * CH_PER_TILE + cc
                    nc.tensor.matmul(
                        v0ps[:],
                        xn16[:, c:c + 1],
                        w0tiles[t][:, cc * 128:(cc + 1) * 128],
                        start=(c == 0),
                        stop=(c == C - 1),
                    )

            v0sb = sp.tile([1, H], FP)
            nc.vector.tensor_copy(v0sb[:], v0ps[:])
            nc.sync.dma_start(outp_d[:], v0sb[:])

    nc.compile()
    return nc


def _build_b():
    """Launch B: fp16 natural slab; reduce partials + chain + J = r0 @ W0."""
    nc = bacc.Bacc("TRN2", target_bir_lowering=False, debug=False,
                   num_devices=N_CORES)

    w0n_d = nc.dram_tensor("w0n", [H, N_LOC], FH, kind="ExternalInput").ap()
    # packed [128, 1048] fp32:
    #   0:128 w1t | 128:256 w2t | 256:384 w3t | 384:512 w1n | 512:640 w2n
    #   | 640:768 w3n | 768:776 bcols (b0,b1,b2,b3,woutT,bout@[0,5])
    #   | 776:784 parts | 784:912 xmaxT | 912:1040 xminT
    sm_d = nc.dram_tensor("sm", [128, 1040], FP, kind="ExternalInput").ap()

    outj_d = nc.dram_tensor("out_j", [128, C], FP, kind="ExternalOutput").ap()
    outv_d = nc.dram_tensor("out_v", [1, 1], FP, kind="ExternalOutput").ap()

    with tile.TileContext(nc) as tc:
        with tc.tile_pool(name="w0", bufs=N_W0_TILES) as w0p, \
             tc.tile_pool(name="small", bufs=1) as sp, \
             tc.tile_pool(name="ps", bufs=2, space="PSUM") as pp, \
             tc.tile_pool(name="psj", bufs=1, space="PSUM") as pjp:

            sm = sp.tile([128, 1040], FP)
            nc.sync.dma_start(sm[:], sm_d[:])
            w0tiles = []
            for t in range(N_W0_TILES):
                w0tile = w0p.tile([128, W0_TILE], FH, tag="w0tile")
                eng = nc.scalar if t % 2 == 0 else nc.sync
                eng.dma_start(w0tile[:], w0n_d[:, t * W0_TILE:(t + 1) * W0_TILE])
                w0tiles.append(w0tile)

            w1t = sm[:, 0:128]
            w2t = sm[:, 128:256]
            w3t = sm[:, 256:384]
            w1n = sm[:, 384:512]
            w2n = sm[:, 512:640]
            w3n = sm[:, 640:768]
            bcols = sm[:, 768:776]
            parts = sm[:, 776:784]
            xmaxT = sm[:, 784:912]
            xminT = sm[:, 912:1040]

            # 1/(xmax-xmin) in [k, c]; the missing *2 is folded into d0.
            xrT = sp.tile([128, C], FP)
            nc.vector.tensor_sub(xrT[:], xmaxT, xminT)
            invT = sp.tile([128, C], FP)
            nc.vector.reciprocal(invT[:], xrT[:])

            # ---- forward chain (vectors as [128, 1] columns) ----
            v0c = sp.tile([H, 1], FP)
            nc.vector.tensor_reduce(v0c[:], parts, mybir.AxisListType.X, AOT.add)

            v1c = sp.tile([H, 1], FP)
            nc.scalar.activation(v1c[:], v0c[:], ACT.Tanh, bias=bcols[:, 0:1])
            d0c = sp.tile([H, 1], FP)  # holds 2*(1 - v1^2)
            nc.vector.tensor_mul(d0c[:], v1c[:], v1c[:])
            nc.vector.tensor_scalar(d0c[:], d0c[:], -2.0, 2.0, AOT.mult, AOT.add)

            a1ps = pp.tile([H, 1], FP, tag="chain_ps")
            nc.tensor.matmul(a1ps[:], w1t, v1c[:], start=True, stop=True)
            v2c = sp.tile([H, 1], FP)
            nc.scalar.activation(v2c[:], a1ps[:], ACT.Tanh, bias=bcols[:, 1:2])
            d1c = sp.tile([H, 1], FP)
            nc.vector.tensor_mul(d1c[:], v2c[:], v2c[:])
            nc.vector.tensor_scalar(d1c[:], d1c[:], -1.0, 1.0, AOT.mult, AOT.add)

            a2ps = pp.tile([H, 1], FP, tag="chain_ps")
            nc.tensor.matmul(a2ps[:], w2t, v2c[:], start=True, stop=True)
            v3c = sp.tile([H, 1], FP)
            nc.scalar.activation(v3c[:], a2ps[:], ACT.Tanh, bias=bcols[:, 2:3])
            d2c = sp.tile([H, 1], FP)
            nc.vector.tensor_mul(d2c[:], v3c[:], v3c[:])
            nc.vector.tensor_scalar(d2c[:], d2c[:], -1.0, 1.0, AOT.mult, AOT.add)

            a3ps = pp.tile([H, 1], FP, tag="chain_ps")
            nc.tensor.matmul(a3ps[:], w3t, v3c[:], start=True, stop=True)
            a3c = sp.tile([H, 1], FP)
            nc.scalar.activation(a3c[:], a3ps[:], ACT.Identity, bias=bcols[:, 3:4])

            voutps = pp.tile([1, 1], FP, tag="chain_ps")
            nc.tensor.matmul(voutps[:], bcols[:, 4:5], a3c[:], start=True, stop=True)
            voutsb = sp.tile([1, 1], FP)
            nc.scalar.activation(voutsb[:], voutps[:], ACT.Identity,
                                 bias=bcols[0:1, 5:6])
            nc.sync.dma_start(outv_d[:], voutsb[:])

            # ---- backward chain, all in column form ----
            # r3 = (Wout @ W3).T = W3.T @ woutT: lhsT = W3 natural.
            r3ps = pp.tile([H, 1], FP, tag="chain_ps")
            nc.tensor.matmul(r3ps[:], w3n, bcols[:, 4:5], start=True, stop=True)
            q2c = sp.tile([H, 1], FP)
            nc.vector.tensor_mul(q2c[:], r3ps[:], d2c[:])

            r2ps = pp.tile([H, 1], FP, tag="chain_ps")
            nc.tensor.matmul(r2ps[:], w2n, q2c[:], start=True, stop=True)
            q1c = sp.tile([H, 1], FP)
            nc.vector.tensor_mul(q1c[:], r2ps[:], d1c[:])

            r1ps = pp.tile([H, 1], FP, tag="chain_ps")
            nc.tensor.matmul(r1ps[:], w1n, q1c[:], start=True, stop=True)
            r0c = sp.tile([H, 1], FP)
            nc.vector.tensor_mul(r0c[:], r1ps[:], d0c[:])
            r016 = sp.tile([H, 1], FH)
            nc.vector.tensor_copy(r016[:], r0c[:])

            # ---- pass 2: J[k, c] = W0_chunk_c.T @ r0 ----
            jt = pjp.tile([128, C], FP)
            for t in range(N_W0_TILES):
                for cc in range(CH_PER_TILE):
                    c = t * CH_PER_TILE + cc
                    nc.tensor.matmul(
                        jt[:, c:c + 1],
                        w0tiles[t][:, cc * 128:(cc + 1) * 128],
                        r016[:],
                        start=True,
                        stop=True,
                    )

            jts = sp.tile([128, C], FP)
            nc.vector.tensor_mul(jts[:], jt[:], invT[:])
            nc.sync.dma_start(outj_d[:], jts[:])

    nc.compile()
    return nc


def _get_kernels():
    if "nc_a" not in _CACHE:
        _CACHE["nc_a"] = _build_a()
        _CACHE["nc_b"] = _build_b()
    return _CACHE["nc_a"], _CACHE["nc_b"]


def kernel(**inputs):
    nc_a, nc_b = _get_kernels()
    f = np.float32

    state = np.asarray(inputs["state"], f).reshape(1, N_STATE)
    x_max = np.asarray(inputs["x_max"], f).reshape(N_STATE)
    x_min = np.asarray(inputs["x_min"], f).reshape(N_STATE)
    W0 = np.asarray(inputs["W0"], f)
    W1 = np.asarray(inputs["W1"], f)
    W2 = np.asarray(inputs["W2"], f)
    W3 = np.asarray(inputs["W3"], f)
    Wout = np.asarray(inputs["Wout"], f).reshape(1, H)
    b0 = np.asarray(inputs["b0"], f).reshape(H)
    b1 = np.asarray(inputs["b1"], f).reshape(H)
    b2 = np.asarray(inputs["b2"], f).reshape(H)
    b3 = np.asarray(inputs["b3"], f).reshape(H)
    bout = np.asarray(inputs["bout"], f).reshape(1)

    xmaxT = []
    xminT = []
    in_maps_a = []
    for i in range(N_CORES):
        sl = slice(i * N_LOC, (i + 1) * N_LOC)
        w0t16 = np.ascontiguousarray(
            W0[:, sl].reshape(H, C, 128).transpose(2, 1, 0)
        ).reshape(128, C * H).astype(np.float16)
        xmaxT.append(np.ascontiguousarray(x_max[sl].reshape(C, 128).T))
        xminT.append(np.ascontiguousarray(x_min[sl].reshape(C, 128).T))
        sm = np.empty((128, 384), f)
        sm[:, 0:128] = state[0, sl].reshape(C, 128).T
        sm[:, 128:256] = xmaxT[i]
        sm[:, 256:384] = xminT[i]
        in_maps_a.append({"w0t": w0t16, "sm": sm})

    trace = bool(int(os.environ.get("KERNEL_TRACE", "0")))
    res_a = bass_utils.run_bass_kernel_spmd(
        nc_a, in_maps_a, core_ids=list(range(N_CORES)), trace=trace
    )
    _CACHE["res_a"] = res_a

    # pure gather: the 8 per-core [1, 128] partial rows -> [128, 8] columns
    parts = np.ascontiguousarray(
        np.concatenate(
            [np.asarray(res_a.results[i]["out_p"]).reshape(H, 1)
             for i in range(N_CORES)], axis=1)
    )

    smb = np.zeros((128, 1040), f)
    smb[:, 0:128] = W1.T
    smb[:, 128:256] = W2.T
    smb[:, 256:384] = W3.T
    smb[:, 384:512] = W1
    smb[:, 512:640] = W2
    smb[:, 640:768] = W3
    smb[:, 768] = b0
    smb[:, 769] = b1
    smb[:, 770] = b2
    smb[:, 771] = b3
    smb[:, 772] = Wout[0]
    smb[0, 773] = bout[0]
    smb[:, 776:784] = parts
    in_maps_b = []
    for i in range(N_CORES):
        sl = slice(i * N_LOC, (i + 1) * N_LOC)
        smi = smb.copy()
        smi[:, 784:912] = xmaxT[i]
        smi[:, 912:1040] = xminT[i]
        in_maps_b.append({
            "w0n": np.ascontiguousarray(W0[:, sl]).astype(np.float16),
            "sm": smi,
        })

    res_b = bass_utils.run_bass_kernel_spmd(
        nc_b, in_maps_b, core_ids=list(range(N_CORES)), trace=trace
    )
    _CACHE["res_b"] = res_b

    out = np.empty((1, N_STATE + 1), np.float32)
    out[0, 0] = float(np.asarray(res_b.results[0]["out_v"]).reshape(()))
    for i in range(N_CORES):
        jt = np.asarray(res_b.results[i]["out_j"])  # [k, c]
        out[0, 1 + i * N_LOC:1 + (i + 1) * N_LOC] = jt.T.reshape(-1)
    return out
